# revision 1
# baseline (speedup 1.0000x reference)
"""Multi-head attention (B=4, L=2048, D=1024, H=16) on 8 NeuronCores.

Sharding: core c handles batch b=c//2 and query rows [1024*(c%2), +1024).
The per-core input x is the batch's [2048, 1024] activations ROTATED so the
core's own query rows are rows 0..1023 (softmax over keys is permutation
invariant, so rotating keys+values together is exact). No collectives needed.

Per-core pipeline (all matmuls in float32r = full-speed ~tf32 precision):
  A)  transpose x -> XT [k, s] (PE transpose); QT = Wq^T@XT[:, :1024],
      KT = Wk^T@XT (SBUF resident); V = XT^T@Wv staged to DRAM with a
      fused ones-column per head (for the softmax denominator).
  B1) per head pair: scores^T tile [s,l] = KT_h^T @ QT_h (contraction d=64,
      row-group paired across the 2 heads); exp via ScalarE (scale=1/8
      folded); PV accumulate [V_h|1]^T @ exp(S^T) -> [65, l] PSUM where row
      64 = softmax denominator; normalize rows 0..63 by broadcasted
      reciprocal.
  C)  y^T = Wo^T @ OT (+bo fused), PE-transpose back to [l, dout], DMA out.
"""

import numpy as np

import sys

for _p in ("/opt/trn_rl_repo", "/opt/pypackages"):
    if _p not in sys.path:
        sys.path.append(_p)

from contextlib import ExitStack

import concourse.bass as bass
import concourse.mybir as mybir
import concourse.tile as tile
from concourse import bacc
from concourse.bass_utils import run_bass_kernel_spmd
from concourse.masks import make_identity

B, L, D, H = 4, 2048, 1024, 16
HD = D // H  # 64
LQ = 1024  # query rows per core
N_CORES = 8
F32 = mybir.dt.float32
F32R = mybir.dt.float32r
AF = mybir.ActivationFunctionType

P = 128
KT_TILES = D // P  # 8 k tiles
ST_TILES = L // P  # 16 s tiles
DT_TILES = D // P  # 8 d tiles
LH = 512  # l half width
SCALE = 1.0 / float(np.sqrt(HD))
PIPELINE = True
COMBINED_EXP = True
B1_LHALF = True
B1_XPAIR = True


def _load_bias(nc, pool, dram, name):
    """[1024] dram vector -> [128, 8] sbuf tile; column t = b[128t:128t+128]."""
    t = pool.tile([P, DT_TILES], F32, name=name)
    nc.gpsimd.dma_start(t[:], dram.rearrange("(t p) -> p t", p=P))
    return t


def build_nc(repeat=1, stop_after=None):
    nc = bacc.Bacc(None)

    x_d = nc.declare_dram_parameter("x", [L, D], F32, isOutput=False)
    wq_d = nc.declare_dram_parameter("wq", [D, D], F32, isOutput=False)
    wk_d = nc.declare_dram_parameter("wk", [D, D], F32, isOutput=False)
    wv_d = nc.declare_dram_parameter("wv", [D, D], F32, isOutput=False)
    wo_d = nc.declare_dram_parameter("wo", [D, D], F32, isOutput=False)
    bq_d = nc.declare_dram_parameter("bq", [D], F32, isOutput=False)
    bk_d = nc.declare_dram_parameter("bk", [D], F32, isOutput=False)
    bv_d = nc.declare_dram_parameter("bv", [D], F32, isOutput=False)
    bo_d = nc.declare_dram_parameter("bo", [D], F32, isOutput=False)
    y_d = nc.declare_dram_parameter("y", [LQ, D], F32, isOutput=True)

    # V staged in DRAM, already augmented with a ones column per head:
    # [s_tile, partition(s), head, 65] where col 64 of each head slot is 1.0
    v_dram = nc.dram_tensor("v_stage", [ST_TILES, P, H, HD + 1], F32R)

    with tile.TileContext(nc) as tc, ExitStack() as ctx:
      for _rep in range(repeat):
       with ExitStack() as rctx:
        singles = rctx.enter_context(tc.tile_pool(name="singles", bufs=1))
        ident32 = singles.tile([P, P], F32, name="ident32")
        make_identity(nc, ident32[:])
        ident = singles.tile([P, P], F32R, name="ident")
        nc.vector.tensor_copy(ident[:], ident32[:])
        bq_sb = _load_bias(nc, singles, bq_d, "bq")
        bk_sb = _load_bias(nc, singles, bk_d, "bk")
        bv_sb = _load_bias(nc, singles, bv_d, "bv")
        bo_sb = _load_bias(nc, singles, bo_d, "bo")

        # big resident slabs
        qt_pool = rctx.enter_context(tc.tile_pool(name="qt", bufs=1))
        kt_pool = rctx.enter_context(tc.tile_pool(name="kt", bufs=1))
        qt = qt_pool.tile([P, DT_TILES, LQ], F32R, name="qt")  # [d%128, dtile, l]
        kt = kt_pool.tile([P, DT_TILES, L], F32R, name="kt")  # [d%128, dtile, s]

        # ---------------- Phase A: transpose + projections ----------------
        with (
            tc.tile_pool(name="xt", bufs=1) as xt_pool,
            tc.tile_pool(name="wpool", bufs=2) as wpool,
            tc.tile_pool(name="vb", bufs=3) as vb_pool,
            tc.tile_pool(name="wv", bufs=1) as wv_pool,
            tc.tile_pool(name="ps_proj", bufs=4, space="PSUM") as ps_proj,
        ):
            xt = xt_pool.tile([P, KT_TILES, L], F32R, name="xt")  # [k%128, ktile, s]

            # transpose x into xt (PE transpose of 128x128 blocks)
            with (
                tc.tile_pool(name="xpool", bufs=3) as xpool,
                tc.tile_pool(name="ps_tr", bufs=3, space="PSUM") as ps_tr,
            ):
                for li in range(ST_TILES):
                    # plain HWDGE fp32 load; fp32->fp32r cast happens for free
                    # in the transpose-evict copy below (4 transposes batched
                    # into one PSUM bank -> single DVE eviction)
                    x_sb = xpool.tile([P, D], F32, name="x_sb")
                    nc.sync.dma_start(x_sb[:], x_d[li * P : (li + 1) * P, :])
                    for kg in range(KT_TILES // 4):
                        pt4 = ps_tr.tile([P, 4, P], F32, name="pt4")
                        for b in range(4):
                            ki = 4 * kg + b
                            nc.tensor.transpose(
                                pt4[:, b, :], x_sb[:, ki * P : (ki + 1) * P], ident32[:]
                            )
                        nc.vector.tensor_copy(
                            xt[:, 4 * kg : 4 * kg + 4, li * P : (li + 1) * P], pt4[:]
                        )

            # QT[d, l] = sum_k Wq[k, d-tile]^T @ XT[k, l]   (+bq fused)
            # KT[d, s] = sum_k Wk[k, d-tile]^T @ XT[k, s]   (+bk fused)
            # W column block per d-tile: [128(k%128), ktile, 128(d)]
            for w_d, b_sb, out_sb, ncols in (
                (wq_d, bq_sb, qt, LQ),
                (wk_d, bk_sb, kt, L),
            ):
                for dt_i in range(DT_TILES):
                    w_col = wpool.tile([P, KT_TILES, P], F32R, name="w_col")
                    nc.gpsimd.dma_start(
                        w_col[:],
                        w_d[:, dt_i * P : (dt_i + 1) * P].rearrange(
                            "(t p) n -> p t n", p=P
                        ),
                    )
                    for ci in range(ncols // LH):
                        ps = ps_proj.tile([P, LH], F32, name="ps_proj")
                        for ki in range(KT_TILES):
                            nc.tensor.matmul(
                                ps[:],
                                w_col[:, ki, :],
                                xt[:, ki, ci * LH : (ci + 1) * LH],
                                start=(ki == 0),
                                stop=(ki == KT_TILES - 1),
                            )
                        nc.scalar.activation(
                            out_sb[:, dt_i, ci * LH : (ci + 1) * LH],
                            ps[:],
                            AF.Identity,
                            bias=b_sb[:, dt_i : dt_i + 1],
                        )

            # V[s, d] = sum_k XT[k, s-tile]^T @ Wv[k, d] staged to DRAM
            # bounce buffer interleaves the per-head ones column.
            for dc in range(2):  # 512-wide chunks = 8 heads each
                wv_half = wv_pool.tile([P, KT_TILES, LH], F32R, name="wv_half")
                nc.gpsimd.dma_start(
                    wv_half[:],
                    wv_d[:, dc * LH : (dc + 1) * LH].rearrange("(t p) n -> p t n", p=P),
                )
                for st in range(ST_TILES):
                    ps = ps_proj.tile([P, LH], F32, name="ps_proj")
                    for ki in range(KT_TILES):
                        nc.tensor.matmul(
                            ps[:],
                            xt[:, ki, st * P : (st + 1) * P],
                            wv_half[:, ki, :],
                            start=(ki == 0),
                            stop=(ki == KT_TILES - 1),
                        )
                    vb = vb_pool.tile([P, 8, HD + 1], F32R, name="vb")
                    nc.vector.memset(vb[:, :, HD : HD + 1].bitcast(F32), 1.0)
                    nc.vector.tensor_copy(vb[:, :, 0:HD], ps[:])
                    nc.sync.dma_start(v_dram[st, :, dc * 8 : (dc + 1) * 8, :], vb[:])

        if stop_after == "a":
            for i in range(KT_TILES):
                nc.sync.dma_start(y_d[i * P : (i + 1) * P, :], qt[:, i, :].bitcast(F32))
            continue

        # ---------------- Phase B1: attention per head pair ----------------
        ot_pool = rctx.enter_context(tc.tile_pool(name="ot", bufs=1))
        ot = ot_pool.tile([P, DT_TILES, LQ], F32R, name="ot")  # [din%128, dintile, l]

        with (
            tc.tile_pool(name="vaug", bufs=2) as vaug_pool,
            tc.tile_pool(name="et", bufs=(5 if B1_LHALF else 3 if COMBINED_EXP else 6)) as et_pool,
            tc.tile_pool(name="otmp", bufs=3) as otmp_pool,
            tc.tile_pool(name="rr", bufs=2) as rr_pool,
            tc.tile_pool(name="rb", bufs=2) as rb_pool,
            tc.tile_pool(name="ps_s", bufs=(2 if B1_LHALF else 1 if COMBINED_EXP else 2), space="PSUM") as ps_s_pool,
            tc.tile_pool(name="ps_o", bufs=2, space="PSUM") as ps_o_pool,
        ):
            if B1_XPAIR:
                # flat unit pipeline across pair boundaries: the lookahead-1
                # scores/exp never drains at a pair boundary
                pair_vaug = {}
                pair_pso = {}

                def ensure_vaug(p):
                    if p not in pair_vaug:
                        v = vaug_pool.tile(
                            [P, ST_TILES, 2 * (HD + 1)], F32R, name="vaug"
                        )
                        nc.sync.dma_start(
                            v[:],
                            v_dram[:, :, 2 * p : 2 * p + 2, :].rearrange(
                                "s p h c -> p s (h c)"
                            ),
                        )
                        pair_vaug[p] = v

                def scores_g(p, st, lh):
                    ps_s = ps_s_pool.tile([P, 2, LH], F32, name="ps_s")
                    for sub in range(2):
                        nc.tensor.matmul(
                            ps_s[:, sub, :],
                            kt[sub * HD : (sub + 1) * HD, p, st * P : (st + 1) * P],
                            qt[sub * HD : (sub + 1) * HD, p, lh * LH : (lh + 1) * LH],
                            start=True,
                            stop=True,
                        )
                    e2 = et_pool.tile([P, 2, LH], F32R, name="et")
                    nc.scalar.activation(e2[:], ps_s[:], AF.Exp, scale=SCALE)
                    return e2

                def pv_g(p, st, lh, e2):
                    if p not in pair_pso:
                        pair_pso[p] = [
                            ps_o_pool.tile([HD + 1, LQ], F32, name="ps_o")
                            for _ in range(2)
                        ]
                    po = pair_pso[p]
                    for sub in range(2):
                        nc.tensor.matmul(
                            po[sub][:, lh * LH : (lh + 1) * LH],
                            pair_vaug[p][:, st, sub * (HD + 1) : (sub + 1) * (HD + 1)],
                            e2[:, sub, :],
                            start=(st == 0),
                            stop=(st == ST_TILES - 1),
                        )

                def epilogue(p):
                    po = pair_pso.pop(p)
                    pair_vaug.pop(p)
                    for sub in range(2):
                        o_tmp = otmp_pool.tile([HD + 1, LQ], F32, name="o_tmp")
                        nc.vector.tensor_copy(o_tmp[:], po[sub][:])
                        r_row = rr_pool.tile([1, LQ], F32, name="r_row")
                        nc.vector.reciprocal(r_row[:], o_tmp[HD : HD + 1, :])
                        r_bc = rb_pool.tile([HD, LQ], F32, name="r_bc")
                        nc.gpsimd.partition_broadcast(r_bc[:], r_row[:])
                        dst = ot[sub * HD : (sub + 1) * HD, p, :]
                        nc.vector.tensor_mul(dst, o_tmp[0:HD, :], r_bc[:])
                        nc.vector.tensor_scalar_add(
                            dst, dst, bv_sb[sub * HD : (sub + 1) * HD, p : p + 1]
                        )

                all_units = [
                    (p, st, lh)
                    for p in range(H // 2)
                    for st in range(ST_TILES)
                    for lh in range(2)
                ]
                prev = None
                for u in all_units:
                    ensure_vaug(u[0])
                    e2 = scores_g(*u)
                    if prev is not None:
                        pv_g(*prev[0], prev[1])
                        if prev[0][1] == ST_TILES - 1 and prev[0][2] == 1:
                            epilogue(prev[0][0])
                    prev = (u, e2)
                pv_g(*prev[0], prev[1])
                epilogue(prev[0][0])

            for pair in ([] if B1_XPAIR else range(H // 2)):
                vaug = vaug_pool.tile([P, ST_TILES, 2 * (HD + 1)], F32R, name="vaug")
                nc.sync.dma_start(
                    vaug[:],
                    v_dram[:, :, 2 * pair : 2 * pair + 2, :].rearrange(
                        "s p h c -> p s (h c)"
                    ),
                )
                ps_o = [
                    ps_o_pool.tile([HD + 1, LQ], F32, name="ps_o") for _ in range(2)
                ]

                def scores_exp_lh(st, lh):
                    # 2-bank scores tile (both subs, one l-half): restores
                    # ps_s double-buffering within the 8-bank PSUM budget
                    ps_s = ps_s_pool.tile([P, 2, LH], F32, name="ps_s")
                    for sub in range(2):
                        nc.tensor.matmul(
                            ps_s[:, sub, :],
                            kt[sub * HD : (sub + 1) * HD, pair, st * P : (st + 1) * P],
                            qt[sub * HD : (sub + 1) * HD, pair, lh * LH : (lh + 1) * LH],
                            start=True,
                            stop=True,
                        )
                    e2 = et_pool.tile([P, 2, LH], F32R, name="et")
                    nc.scalar.activation(e2[:], ps_s[:], AF.Exp, scale=SCALE)
                    return e2

                def pv_lh(st, lh, e2):
                    for sub in range(2):
                        nc.tensor.matmul(
                            ps_o[sub][:, lh * LH : (lh + 1) * LH],
                            vaug[:, st, sub * (HD + 1) : (sub + 1) * (HD + 1)],
                            e2[:, sub, :],
                            start=(st == 0),
                            stop=(st == ST_TILES - 1),
                        )

                def scores_exp(st):
                    if COMBINED_EXP:
                        # both heads' scores into one 4-bank PSUM tile so a
                        # SINGLE [128, 2048] ACTIVATE covers them (halves the
                        # per-op ScalarE overhead)
                        ps_s = ps_s_pool.tile([P, 2, LQ], F32, name="ps_s")
                        for sub in range(2):
                            for lh in range(2):
                                nc.tensor.matmul(
                                    ps_s[:, sub, lh * LH : (lh + 1) * LH],
                                    kt[sub * HD : (sub + 1) * HD, pair, st * P : (st + 1) * P],
                                    qt[sub * HD : (sub + 1) * HD, pair, lh * LH : (lh + 1) * LH],
                                    start=True,
                                    stop=True,
                                )
                        e2 = et_pool.tile([P, 2, LQ], F32R, name="et")
                        nc.scalar.activation(e2[:], ps_s[:], AF.Exp, scale=SCALE)
                        return [e2[:, 0, :], e2[:, 1, :]]
                    et = [None, None]
                    for sub in range(2):
                        ps_s = ps_s_pool.tile([P, LQ], F32, name="ps_s")
                        for lh in range(2):
                            nc.tensor.matmul(
                                ps_s[:, lh * LH : (lh + 1) * LH],
                                kt[sub * HD : (sub + 1) * HD, pair, st * P : (st + 1) * P],
                                qt[sub * HD : (sub + 1) * HD, pair, lh * LH : (lh + 1) * LH],
                                start=True,
                                stop=True,
                            )
                        e = et_pool.tile([P, LQ], F32R, name="et")
                        nc.scalar.activation(e[:], ps_s[:], AF.Exp, scale=SCALE)
                        et[sub] = e
                    return et

                def pv(st, et):
                    for sub in range(2):
                        for lh in range(2):
                            nc.tensor.matmul(
                                ps_o[sub][:, lh * LH : (lh + 1) * LH],
                                vaug[:, st, sub * (HD + 1) : (sub + 1) * (HD + 1)],
                                et[sub][:, lh * LH : (lh + 1) * LH],
                                start=(st == 0),
                                stop=(st == ST_TILES - 1),
                            )

                if B1_LHALF:
                    units = [(st, lh) for st in range(ST_TILES) for lh in range(2)]
                    e_cur = scores_exp_lh(*units[0])
                    for i, u in enumerate(units):
                        e_next = (
                            scores_exp_lh(*units[i + 1]) if i + 1 < len(units) else None
                        )
                        pv_lh(*u, e_cur)
                        e_cur = e_next
                elif PIPELINE:
                    # software pipeline: scores(st+1) emitted before pv(st) so
                    # PE has independent work while ACT computes exp(st)
                    et_cur = scores_exp(0)
                    for st in range(ST_TILES):
                        et_next = scores_exp(st + 1) if st + 1 < ST_TILES else None
                        pv(st, et_cur)
                        et_cur = et_next
                else:
                    for st in range(ST_TILES):
                        pv(st, scores_exp(st))
                # evict O+denominator to SBUF immediately (frees the PSUM
                # bank for the next pair), then normalize rows 0..63 by the
                # broadcasted reciprocal of row 64, write into ot slab (+bv).
                for sub in range(2):
                    o_tmp = otmp_pool.tile([HD + 1, LQ], F32, name="o_tmp")
                    nc.vector.tensor_copy(o_tmp[:], ps_o[sub][:])
                    r_row = rr_pool.tile([1, LQ], F32, name="r_row")
                    nc.vector.reciprocal(r_row[:], o_tmp[HD : HD + 1, :])
                    r_bc = rb_pool.tile([HD, LQ], F32, name="r_bc")
                    nc.gpsimd.partition_broadcast(r_bc[:], r_row[:])
                    dst = ot[sub * HD : (sub + 1) * HD, pair, :]
                    nc.vector.tensor_mul(dst, o_tmp[0:HD, :], r_bc[:])
                    nc.vector.tensor_scalar_add(
                        dst, dst, bv_sb[sub * HD : (sub + 1) * HD, pair : pair + 1]
                    )

        if stop_after == "ab":
            for i in range(KT_TILES):
                nc.sync.dma_start(y_d[i * P : (i + 1) * P, :], ot[:, i, :].bitcast(F32))
            continue

        # ---------------- Phase C: output projection + transpose ----------------
        with (
            tc.tile_pool(name="wo", bufs=2) as wo_pool,
            tc.tile_pool(name="gt", bufs=2) as gt_pool,
            tc.tile_pool(name="ysl", bufs=1) as y_pool,
            tc.tile_pool(name="ps_g", bufs=2, space="PSUM") as ps_g_pool,
            tc.tile_pool(name="ps_t", bufs=3, space="PSUM") as ps_t_pool,
        ):
            y_sb = y_pool.tile([P, KT_TILES, D], F32, name="y_sb")  # [l%128, ltile, dout]
            for j in range(DT_TILES):  # dout tiles
                wo_sb = wo_pool.tile([P, KT_TILES, P], F32R, name="wo_sb")
                nc.gpsimd.dma_start(
                    wo_sb[:],
                    wo_d[:, j * P : (j + 1) * P].rearrange("(t p) n -> p t n", p=P),
                )
                gt_s = gt_pool.tile([P, LQ], F32R, name="gt_s")
                for lh in range(2):
                    ps_g = ps_g_pool.tile([P, LH], F32, name="ps_g")
                    for ki in range(KT_TILES):
                        nc.tensor.matmul(
                            ps_g[:],
                            wo_sb[:, ki, :],
                            ot[:, ki, lh * LH : (lh + 1) * LH],
                            start=(ki == 0),
                            stop=(ki == KT_TILES - 1),
                        )
                    nc.scalar.activation(
                        gt_s[:, lh * LH : (lh + 1) * LH],
                        ps_g[:],
                        AF.Identity,
                        bias=bo_sb[:, j : j + 1],
                    )
                for a in range(KT_TILES // 4):  # l tiles, batched 4-per-bank
                    pt4 = ps_t_pool.tile([P, 4, P], F32R, name="pt4_out")
                    for b in range(4):
                        i = 4 * a + b
                        nc.tensor.transpose(
                            pt4[:, b, :], gt_s[:, i * P : (i + 1) * P], ident[:]
                        )
                    nc.vector.tensor_copy(
                        y_sb[:, 4 * a : 4 * a + 4, j * P : (j + 1) * P], pt4[:]
                    )
            for i in range(KT_TILES):
                nc.sync.dma_start(y_d[i * P : (i + 1) * P, :], y_sb[:, i, :])

    nc.finalize()
    return nc


_NC_CACHE = None


def kernel(**inputs):
    global _NC_CACHE
    if _NC_CACHE is None:
        _NC_CACHE = build_nc()
    nc = _NC_CACHE

    q = np.ascontiguousarray(np.asarray(inputs["q"], dtype=np.float32))
    w = {k: np.ascontiguousarray(np.asarray(inputs[k], dtype=np.float32))
         for k in ("Wq", "Wk", "Wv", "Wo", "bq", "bk", "bv", "bo")}

    in_maps = []
    for c in range(N_CORES):
        b, half = c // 2, c % 2
        lo = LQ * half
        x_rot = np.concatenate([q[b, lo:], q[b, :lo]], axis=0)
        in_maps.append({
            "x": np.ascontiguousarray(x_rot),
            "wq": w["Wq"], "wk": w["Wk"], "wv": w["Wv"], "wo": w["Wo"],
            "bq": w["bq"], "bk": w["bk"], "bv": w["bv"], "bo": w["bo"],
        })

    res = run_bass_kernel_spmd(nc, in_maps, core_ids=list(range(N_CORES)))

    out = np.empty((B, L, D), dtype=np.float32)
    for c in range(N_CORES):
        b, half = c // 2, c % 2
        lo = LQ * half
        out[b, lo : lo + LQ, :] = res.results[c]["y"]
    return out

